# revision 11
# baseline (speedup 1.0000x reference)
"""Trainium2 Bass kernel: GroupNorm + single-head self-attention block.

Reference computation (per batch b):
    xn = GroupNorm(x, 16 groups, eps=1e-5) * gamma + beta
    q/k/v = W @ xn + b          (1x1 conv == channel matmul), [C, N]
    S = (q^T k) / sqrt(C)       [N, N]
    A = softmax_j(S)
    O = v @ A^T                 [C, N]
    y = wo @ O + bo + x

Shapes: B=4, C=256, H=W=64 -> N=4096.

Sharding: 8 cores = 4 batches x 2 query-halves.  Each core receives the
full x[b] with its query half permuted to the front, computes xn / v
for all N keys (cheap, avoids any collectives) and runs attention for
its 2048 queries.  The device program is identical on all cores (SPMD).

Device algorithm (per core):
  - GroupNorm stats via bn_stats/bn_aggr per channel + PE matmul with a
    group-indicator matrix for the cross-partition (channel) reduction.
  - Scores are computed TRANSPOSED: S^T[j, i] = sum_c k[c,j] q[c,i], so
    both matmul operands are natural [C, *] layouts (no transposes).
    k is never materialized: S^T = xn^T @ (wk^T q), and the bk bias is
    dropped -- softmax over keys is invariant to a per-query shift.
  - exp without max-subtraction (scores ~ N(0,1); fp32 exp is safe).
  - softmax denominator: ones-vector matmul over partitions on PE.
  - O = v @ A^T with v materialized transposed ([N, C]) directly off
    the projection matmul, so A^T (= exp(S^T)) is consumed in place.
  - normalization (1/sum) is applied after the wo projection via a
    PE-broadcast of the reciprocal row.

Big matmuls run in float32r (full-rate fp32 PE mode).  fp32r operands
must be produced "rounded" by a compute engine, so every matmul input
tile is written by DVE/ACT with a float32r output dtype.
"""

import sys

sys.path.insert(0, "/opt/trn_rl_repo")

from contextlib import ExitStack

import numpy as np

import concourse.bass as bass
import concourse.bacc as bacc
import concourse.mybir as mybir
import concourse.tile as tile

B, C, H, W = 4, 256, 64, 64
N = H * W              # keys per batch
GROUPS = 16
EPS = 1e-5
NCORES = 8
QSPLIT = NCORES // B   # query shards per batch
NQ = N // QSPLIT       # queries per core
P = 128
CCH = C // P           # channel chunks (2)
IB = 512               # query block (one PSUM bank of f32)
NIB = NQ // IB         # query blocks per core
NJT = N // P           # key tiles (32)
GSZ = C // GROUPS      # channels per group (16)

F32 = mybir.dt.float32
F32R = mybir.dt.float32r
AF = mybir.ActivationFunctionType
OP = mybir.AluOpType


def build_nc(mm_fast: bool = True):
    """Emit the single-core SPMD program."""
    DTM = F32R if mm_fast else F32   # matmul operand dtype
    nc = bacc.Bacc()

    x_d = nc.declare_dram_parameter("x", [C, N], F32, isOutput=False)
    wqT_d = nc.declare_dram_parameter("wqT", [C, C], F32, isOutput=False)
    wk_d = nc.declare_dram_parameter("wk", [C, C], F32, isOutput=False)
    wvT_d = nc.declare_dram_parameter("wvT", [C, C], F32, isOutput=False)
    woT_d = nc.declare_dram_parameter("woT", [C, C], F32, isOutput=False)
    gamma_d = nc.declare_dram_parameter("gamma", [C], F32, isOutput=False)
    beta_d = nc.declare_dram_parameter("beta", [C], F32, isOutput=False)
    bq_d = nc.declare_dram_parameter("bq", [C], F32, isOutput=False)
    bv_d = nc.declare_dram_parameter("bv", [C], F32, isOutput=False)
    bo_d = nc.declare_dram_parameter("bo", [C], F32, isOutput=False)
    gind_d = nc.declare_dram_parameter("gind", [CCH, P, GROUPS], F32, isOutput=False)
    gindT_d = nc.declare_dram_parameter("gindT", [CCH, GROUPS, P], F32, isOutput=False)
    y_d = nc.declare_dram_parameter("y", [C, NQ], F32, isOutput=True)

    with tile.TileContext(nc) as tc, ExitStack() as ctx:
        const = ctx.enter_context(tc.tile_pool(name="const", bufs=1))
        data = ctx.enter_context(tc.tile_pool(name="data", bufs=1))

        # ---- weights: DMA to f32 staging, DVE-copy to fp32r tiles ----
        stage = ctx.enter_context(tc.tile_pool(name="stage", bufs=1))

        def load_w(handle, nm):
            tiles = []
            for ch in range(CCH):
                s = stage.tile([P, C], F32, name=f"{nm}{ch}_s", tag=f"{nm}{ch}_s")
                nc.sync.dma_start(out=s, in_=handle[ch * P:(ch + 1) * P, :])
                t = const.tile([P, C], DTM, name=f"{nm}{ch}")
                nc.vector.tensor_copy(t, s)
                tiles.append(t)
            return tiles

        wqT = load_w(wqT_d, "wqT")
        wk = load_w(wk_d, "wk")      # used as lhsT directly: (wk^T q)
        wvT = load_w(wvT_d, "wvT")
        woT = load_w(woT_d, "woT")

        def load_vec(handle, nm):
            tiles = []
            for ch in range(CCH):
                t = const.tile([P, 1], F32, name=f"{nm}{ch}")
                nc.sync.dma_start(
                    out=t, in_=handle[ch * P:(ch + 1) * P].unsqueeze(1)
                )
                tiles.append(t)
            return tiles

        gamma = load_vec(gamma_d, "gamma")
        beta = load_vec(beta_d, "beta")
        bq = load_vec(bq_d, "bq")
        bo = load_vec(bo_d, "bo")

        bv_s = stage.tile([1, C], F32, name="bv_s")
        nc.sync.dma_start(out=bv_s, in_=bv_d[:].unsqueeze(0))
        bv_row = const.tile([1, C], DTM, name="bv_row")
        nc.vector.tensor_copy(bv_row, bv_s)

        gind = []
        gindT = []
        for ch in range(CCH):
            gi = const.tile([P, GROUPS], F32, name=f"gind{ch}")
            nc.sync.dma_start(out=gi, in_=gind_d[ch])
            gind.append(gi)
            gt = const.tile([GROUPS, P], F32, name=f"gindT{ch}")
            nc.sync.dma_start(out=gt, in_=gindT_d[ch])
            gindT.append(gt)

        # fp32r lhsT free-dim counts must be even -> ones "column" is [P, 2]
        # (memset cannot emit fp32r; stage in f32 and DVE-copy to round)
        ones_f = const.tile([P, P], F32, name="ones_f")
        nc.vector.memset(ones_f, 1.0)
        ones_col2 = const.tile([P, 2], DTM, name="ones_col2")
        nc.vector.tensor_copy(ones_col2, ones_f[:, 0:2])
        ones_row_r = const.tile([1, P], DTM, name="ones_row_r")
        nc.vector.tensor_copy(ones_row_r, ones_f[0:1, :])
        ones_row_f = ones_f[0:1, :]

        # ---- x in (staging pool released after GroupNorm) ----
        xn = data.tile([P, CCH, N], DTM, name="xn")
        resid = data.tile([P, CCH, NQ], F32, name="resid")

        with tc.tile_pool(name="xf_pool", bufs=1) as xf_pool, \
             tc.tile_pool(name="gn_psum", bufs=1, space="PSUM") as gn_psum, \
             tc.tile_pool(name="gn_sb", bufs=1) as gn_sb:
            xf = xf_pool.tile([P, CCH, N], F32, name="xf")
            for ch in range(CCH):
                nc.sync.dma_start(
                    out=xf[:, ch, :], in_=x_d[ch * P:(ch + 1) * P, :]
                )
            # residual (+ bo) for the local query half
            for ch in range(CCH):
                nc.scalar.activation(
                    out=resid[:, ch, :], in_=xf[:, ch, :NQ], func=AF.Identity,
                    bias=bo[ch], scale=1.0,
                )

            # ---- GroupNorm stats ----
            NS = N // 512  # bn_stats subgroups
            pc = []  # per-channel [mean, mean^2 + var] per chunk
            for ch in range(CCH):
                st6 = gn_sb.tile([P, NS, 6], F32, name=f"st6_{ch}")
                for sg in range(NS):
                    nc.vector.bn_stats(
                        out=st6[:, sg, :], in_=xf[:, ch, sg * 512:(sg + 1) * 512]
                    )
                mv = gn_sb.tile([P, 2], F32, name=f"mv{ch}")
                nc.vector.bn_aggr(out=mv, in_=st6)
                pcs = gn_sb.tile([P, 2], F32, name=f"pcs{ch}")
                nc.vector.tensor_copy(pcs[:, 0:1], mv[:, 0:1])
                # pcs[:,1] = mean^2 + var  (-> group E[x^2] after averaging)
                msq = gn_sb.tile([P, 1], F32, name=f"msq{ch}")
                nc.vector.tensor_mul(msq, mv[:, 0:1], mv[:, 0:1])
                nc.vector.tensor_add(pcs[:, 1:2], mv[:, 1:2], msq)
                pc.append(pcs)

            gs_ps = gn_psum.tile([GROUPS, 2], F32, name="gs_ps")
            for ch in range(CCH):
                nc.tensor.matmul(
                    gs_ps, lhsT=gind[ch], rhs=pc[ch],
                    start=(ch == 0), stop=(ch == CCH - 1),
                )
            # per-channel stats are already means -> average over the GSZ
            # channels of each group
            gs = gn_sb.tile([GROUPS, 2], F32, name="gs")
            nc.scalar.mul(gs, gs_ps, 1.0 / GSZ)
            gvar = gn_sb.tile([GROUPS, 1], F32, name="gvar")
            gmsq = gn_sb.tile([GROUPS, 1], F32, name="gmsq")
            nc.vector.tensor_mul(gmsq, gs[:, 0:1], gs[:, 0:1])
            nc.vector.tensor_sub(gvar, gs[:, 1:2], gmsq)
            # rstd = 1/sqrt(var+eps)
            gstd = gn_sb.tile([GROUPS, 1], F32, name="gstd")
            eps_t = gn_sb.tile([GROUPS, 1], F32, name="eps_t")
            nc.vector.memset(eps_t, EPS)
            nc.scalar.activation(
                out=gstd, in_=gvar, func=AF.Sqrt, bias=eps_t, scale=1.0
            )
            gmr = gn_sb.tile([GROUPS, 2], F32, name="gmr")
            nc.vector.tensor_copy(gmr[:, 0:1], gs[:, 0:1])
            nc.vector.reciprocal(gmr[:, 1:2], gstd)

            # broadcast group (mean, rstd) back to channels, build affine
            for ch in range(CCH):
                cb_ps = gn_psum.tile([P, 2], F32, name="cb_ps", tag="cb_ps")
                nc.tensor.matmul(cb_ps, lhsT=gindT[ch], rhs=gmr,
                                 start=True, stop=True)
                cb = gn_sb.tile([P, 2], F32, name=f"cb{ch}")
                nc.vector.tensor_copy(cb, cb_ps)
                scale = gn_sb.tile([P, 1], F32, name=f"scale{ch}")
                nc.vector.tensor_mul(scale, gamma[ch], cb[:, 1:2])
                shift = gn_sb.tile([P, 1], F32, name=f"shift{ch}")
                nc.vector.tensor_mul(shift, cb[:, 0:1], scale)
                nc.vector.tensor_sub(shift, beta[ch], shift)
                # xn = x * scale + shift
                nc.vector.tensor_scalar(
                    out=xn[:, ch, :], in0=xf[:, ch, :],
                    scalar1=scale, scalar2=shift, op0=OP.mult, op1=OP.add,
                )

        # ---- projections ----
        q = data.tile([P, CCH, NQ], DTM, name="q")      # wq^T xn + bq
        qk = data.tile([P, CCH, NQ], DTM, name="qk")    # wk^T q
        vT = data.tile([P, NJT, C], DTM, name="vT")     # v transposed [N, C]

        with tc.tile_pool(name="pj_psum", bufs=3, space="PSUM") as pj_psum:
            # q[o, i] = sum_c wqT[c, o] xn[c, i] + bq[o]
            for oc in range(CCH):
                for it in range(NQ // 512):
                    ps = pj_psum.tile([P, 512], F32, name="q_ps", tag="q_ps")
                    for ch in range(CCH):
                        nc.tensor.matmul(
                            ps,
                            lhsT=wqT[ch][:, oc * P:(oc + 1) * P],
                            rhs=xn[:, ch, it * 512:(it + 1) * 512],
                            start=(ch == 0), stop=(ch == CCH - 1),
                        )
                    nc.vector.tensor_scalar_add(
                        q[:, oc, it * 512:(it + 1) * 512], ps, scalar1=bq[oc]
                    )
            # qk[c', i] = sum_o wk[o, c'] q[o, i]
            for oc in range(CCH):
                for it in range(NQ // 512):
                    ps = pj_psum.tile([P, 512], F32, name="qk_ps", tag="q_ps")
                    for ch in range(CCH):
                        nc.tensor.matmul(
                            ps,
                            lhsT=wk[ch][:, oc * P:(oc + 1) * P],
                            rhs=q[:, ch, it * 512:(it + 1) * 512],
                            start=(ch == 0), stop=(ch == CCH - 1),
                        )
                    nc.vector.tensor_copy(qk[:, oc, it * 512:(it + 1) * 512], ps)
            # vT[j, c] = sum_c' xn[c', j] wvT[c', c] + bv[c]
            for jt in range(NJT):
                ps = pj_psum.tile([P, C], F32, name="vT_ps", tag="vT_ps")
                for ch in range(CCH):
                    nc.tensor.matmul(
                        ps,
                        lhsT=xn[:, ch, jt * P:(jt + 1) * P],
                        rhs=wvT[ch],
                        start=(ch == 0), stop=False,
                    )
                nc.tensor.matmul(
                    ps, lhsT=ones_row_r, rhs=bv_row,
                    start=False, stop=True,
                )
                nc.vector.tensor_copy(vT[:, jt, :], ps)

        # ---- attention ----
        with tc.tile_pool(name="st_psum", bufs=3, space="PSUM") as st_psum, \
             tc.tile_pool(name="o_psum", bufs=1, space="PSUM") as o_psum, \
             tc.tile_pool(name="sm_psum", bufs=1, space="PSUM") as sm_psum, \
             tc.tile_pool(name="at_pool", bufs=4) as at_pool, \
             tc.tile_pool(name="fin", bufs=2) as fin:
            for ib in range(NIB):
                isl = slice(ib * IB, (ib + 1) * IB)
                sums_ps = sm_psum.tile([2, IB], F32, name="sums_ps", tag="sums")
                o_ps = [
                    o_psum.tile([P, IB], F32, name=f"o_ps{cc}", tag=f"o{cc}")
                    for cc in range(CCH)
                ]
                for jt in range(NJT):
                    jsl = slice(jt * P, (jt + 1) * P)
                    st = st_psum.tile([P, IB], F32, name="st", tag="st")
                    # S^T[j, i] = sum_c' xn[c', j] qk[c', i]
                    for ch in range(CCH):
                        nc.tensor.matmul(
                            st,
                            lhsT=xn[:, ch, jsl],
                            rhs=qk[:, ch, isl],
                            start=(ch == 0), stop=(ch == CCH - 1),
                        )
                    at = at_pool.tile([P, IB], DTM, name="at", tag="at")
                    # A^T = exp(S^T / sqrt(C))
                    nc.scalar.activation(
                        out=at, in_=st, func=AF.Exp, scale=1.0 / 16.0
                    )
                    # denominator: column sums via ones-matmul (row 0; row 1
                    # is a duplicate forced by the even-free-dim rule)
                    nc.tensor.matmul(
                        sums_ps, lhsT=ones_col2, rhs=at,
                        start=(jt == 0), stop=(jt == NJT - 1),
                    )
                    # O[c, i] += v^T[j, c]^T A^T[j, i]
                    for cc in range(CCH):
                        nc.tensor.matmul(
                            o_ps[cc],
                            lhsT=vT[:, jt, cc * P:(cc + 1) * P],
                            rhs=at,
                            start=(jt == 0), stop=(jt == NJT - 1),
                        )

                recip = fin.tile([1, IB], F32, name="recip", tag="recip")
                nc.vector.reciprocal(recip, sums_ps[0:1, :])
                rb_ps = sm_psum.tile([P, IB], F32, name="rb_ps", tag="rb")
                nc.tensor.matmul(rb_ps, lhsT=ones_row_f, rhs=recip,
                                 start=True, stop=True)
                rb = fin.tile([P, IB], F32, name="rb", tag="rbs")
                nc.scalar.copy(rb, rb_ps)

                o_sb = []
                for cc in range(CCH):
                    t = fin.tile([P, IB], DTM, name=f"o_sb{cc}", tag=f"osb{cc}")
                    nc.vector.tensor_copy(t, o_ps[cc])
                    o_sb.append(t)

                for oc in range(CCH):
                    op_ps = st_psum.tile([P, IB], F32, name="op_ps", tag="st")
                    for cc in range(CCH):
                        nc.tensor.matmul(
                            op_ps,
                            lhsT=woT[cc][:, oc * P:(oc + 1) * P],
                            rhs=o_sb[cc],
                            start=(cc == 0), stop=(cc == CCH - 1),
                        )
                    t = fin.tile([P, IB], F32, name="t_sb", tag="t_sb")
                    nc.vector.tensor_mul(t, op_ps, rb)
                    out_sb = fin.tile([P, IB], F32, name="out_sb", tag="out_sb")
                    nc.vector.tensor_add(out_sb, t, resid[:, oc, isl])
                    nc.sync.dma_start(
                        out=y_d[oc * P:(oc + 1) * P, isl], in_=out_sb
                    )
    nc.finalize()
    return nc


_NC_CACHE = {}


def _get_nc(mm_fast=True):
    key = mm_fast
    if key not in _NC_CACHE:
        _NC_CACHE[key] = build_nc(mm_fast)
    return _NC_CACHE[key]


def make_in_maps(inputs):
    """Shard full inputs into per-core input maps."""
    x = np.asarray(inputs["x"], np.float32).reshape(B, C, N)
    gamma = np.asarray(inputs["gamma"], np.float32)
    beta = np.asarray(inputs["beta"], np.float32)
    wq = np.asarray(inputs["wq"], np.float32)
    bq = np.asarray(inputs["bq"], np.float32)
    wk = np.asarray(inputs["wk"], np.float32)
    wv = np.asarray(inputs["wv"], np.float32)
    bv = np.asarray(inputs["bv"], np.float32)
    wo = np.asarray(inputs["wo"], np.float32)
    bo = np.asarray(inputs["bo"], np.float32)

    gind = np.zeros((CCH, P, GROUPS), np.float32)
    for ch in range(CCH):
        for p in range(P):
            gind[ch, p, (ch * P + p) // GSZ] = 1.0
    gindT = np.ascontiguousarray(gind.transpose(0, 2, 1))

    shared = {
        "wqT": np.ascontiguousarray(wq.T),
        "wk": np.ascontiguousarray(wk),
        "wvT": np.ascontiguousarray(wv.T),
        "woT": np.ascontiguousarray(wo.T),
        "gamma": gamma, "beta": beta,
        "bq": bq, "bv": bv, "bo": bo,
        "gind": gind, "gindT": gindT,
    }
    in_maps = []
    for core in range(NCORES):
        b, h = divmod(core, QSPLIT)
        if h == 0:
            xc = x[b]
        else:
            xc = np.concatenate(
                [x[b][:, h * NQ:(h + 1) * NQ], x[b][:, :h * NQ],
                 x[b][:, (h + 1) * NQ:]], axis=1,
            )
        in_maps.append({"x": np.ascontiguousarray(xc), **shared})
    return in_maps


def gather_output(results):
    y = np.empty((B, C, N), np.float32)
    for core in range(NCORES):
        b, h = divmod(core, QSPLIT)
        y[b][:, h * NQ:(h + 1) * NQ] = results[core]["y"]
    return y.reshape(B, C, H, W)


def run_spmd(inputs, trace=False, mm_fast=True):
    from concourse.bass_utils import run_bass_kernel_spmd

    nc = _get_nc(mm_fast)
    in_maps = make_in_maps(inputs)
    res = run_bass_kernel_spmd(
        nc, in_maps, list(range(NCORES)), trace=trace
    )
    return gather_output(res.results), res


def kernel(**inputs) -> np.ndarray:
    out, _ = run_spmd(inputs, trace=False)
    return out


# revision 12
# speedup vs baseline: 8261.2194x; 8261.2194x over previous
"""Trainium2 Bass kernel: GroupNorm + single-head self-attention block.

Reference computation (per batch b):
    xn = GroupNorm(x, 16 groups, eps=1e-5) * gamma + beta
    q/k/v = W @ xn + b          (1x1 conv == channel matmul), [C, N]
    S = (q^T k) / sqrt(C)       [N, N]
    A = softmax_j(S)
    O = v @ A^T                 [C, N]
    y = wo @ O + bo + x

Shapes: B=4, C=256, H=W=64 -> N=4096.

Sharding: 8 cores = 4 batches x 2 query-halves.  Each core receives the
full x[b] with its query half permuted to the front, computes xn / v
for all N keys (cheap, avoids any collectives) and runs attention for
its 2048 queries.  The device program is identical on all cores (SPMD).

Device algorithm (per core):
  - GroupNorm stats via bn_stats/bn_aggr per channel + PE matmul with a
    group-indicator matrix for the cross-partition (channel) reduction.
  - Scores are computed TRANSPOSED: S^T[j, i] = sum_c k[c,j] q[c,i], so
    both matmul operands are natural [C, *] layouts (no transposes).
    k is never materialized: S^T = xn^T @ (wk^T q), and the bk bias is
    dropped -- softmax over keys is invariant to a per-query shift.
  - exp without max-subtraction (scores ~ N(0,1); fp32 exp is safe).
  - softmax denominator: ones-vector matmul over partitions on PE.
  - O = v @ A^T with v materialized transposed ([N, C]) directly off
    the projection matmul, so A^T (= exp(S^T)) is consumed in place.
  - normalization (1/sum) is applied after the wo projection via a
    PE-broadcast of the reciprocal row.

Big matmuls run in float32r (full-rate fp32 PE mode).  fp32r operands
must be produced "rounded" by a compute engine, so every matmul input
tile is written by DVE/ACT with a float32r output dtype.
"""

import sys

sys.path.insert(0, "/opt/trn_rl_repo")

from contextlib import ExitStack

import numpy as np

import concourse.bass as bass
import concourse.bacc as bacc
import concourse.mybir as mybir
import concourse.tile as tile

B, C, H, W = 4, 256, 64, 64
N = H * W              # keys per batch
GROUPS = 16
EPS = 1e-5
NCORES = 8
QSPLIT = NCORES // B   # query shards per batch
NQ = N // QSPLIT       # queries per core
P = 128
CCH = C // P           # channel chunks (2)
IB = 512               # query block (one PSUM bank of f32)
NIB = NQ // IB         # query blocks per core
NJT = N // P           # key tiles (32)
GSZ = C // GROUPS      # channels per group (16)

F32 = mybir.dt.float32
F32R = mybir.dt.float32r
AF = mybir.ActivationFunctionType
OP = mybir.AluOpType


def build_nc(mm_fast: bool = True):
    """Emit the single-core SPMD program."""
    DTM = F32R if mm_fast else F32   # matmul operand dtype
    nc = bacc.Bacc()

    x_d = nc.declare_dram_parameter("x", [C, N], F32, isOutput=False)
    wqT_d = nc.declare_dram_parameter("wqT", [C, C], F32, isOutput=False)
    wk_d = nc.declare_dram_parameter("wk", [C, C], F32, isOutput=False)
    wvT_d = nc.declare_dram_parameter("wvT", [C, C], F32, isOutput=False)
    woT_d = nc.declare_dram_parameter("woT", [C, C], F32, isOutput=False)
    gamma_d = nc.declare_dram_parameter("gamma", [C], F32, isOutput=False)
    beta_d = nc.declare_dram_parameter("beta", [C], F32, isOutput=False)
    bq_d = nc.declare_dram_parameter("bq", [C], F32, isOutput=False)
    bv_d = nc.declare_dram_parameter("bv", [C], F32, isOutput=False)
    bo_d = nc.declare_dram_parameter("bo", [C], F32, isOutput=False)
    gind_d = nc.declare_dram_parameter("gind", [CCH, P, GROUPS], F32, isOutput=False)
    gindT_d = nc.declare_dram_parameter("gindT", [CCH, GROUPS, P], F32, isOutput=False)
    y_d = nc.declare_dram_parameter("y", [C, NQ], F32, isOutput=True)

    with tile.TileContext(nc) as tc, ExitStack() as ctx:
        const = ctx.enter_context(tc.tile_pool(name="const", bufs=1))
        data = ctx.enter_context(tc.tile_pool(name="data", bufs=1))

        # ---- weights: DMA to f32 staging, DVE-copy to fp32r tiles ----
        stage = ctx.enter_context(tc.tile_pool(name="stage", bufs=1))

        def load_w(handle, nm):
            tiles = []
            for ch in range(CCH):
                s = stage.tile([P, C], F32, name=f"{nm}{ch}_s", tag=f"{nm}{ch}_s")
                nc.sync.dma_start(out=s, in_=handle[ch * P:(ch + 1) * P, :])
                t = const.tile([P, C], DTM, name=f"{nm}{ch}")
                nc.vector.tensor_copy(t, s)
                tiles.append(t)
            return tiles

        wqT = load_w(wqT_d, "wqT")
        wk = load_w(wk_d, "wk")      # used as lhsT directly: (wk^T q)
        wvT = load_w(wvT_d, "wvT")
        woT = load_w(woT_d, "woT")

        def load_vec(handle, nm):
            tiles = []
            for ch in range(CCH):
                t = const.tile([P, 1], F32, name=f"{nm}{ch}")
                nc.sync.dma_start(
                    out=t, in_=handle[ch * P:(ch + 1) * P].unsqueeze(1)
                )
                tiles.append(t)
            return tiles

        gamma = load_vec(gamma_d, "gamma")
        beta = load_vec(beta_d, "beta")
        bq = load_vec(bq_d, "bq")
        bo = load_vec(bo_d, "bo")

        bv_s = stage.tile([1, C], F32, name="bv_s")
        nc.sync.dma_start(out=bv_s, in_=bv_d[:].unsqueeze(0))
        bv_row = const.tile([1, C], DTM, name="bv_row")
        nc.vector.tensor_copy(bv_row, bv_s)

        gind = []
        gindT = []
        for ch in range(CCH):
            gi = const.tile([P, GROUPS], F32, name=f"gind{ch}")
            nc.sync.dma_start(out=gi, in_=gind_d[ch])
            gind.append(gi)
            gt = const.tile([GROUPS, P], F32, name=f"gindT{ch}")
            nc.sync.dma_start(out=gt, in_=gindT_d[ch])
            gindT.append(gt)

        # fp32r lhsT free-dim counts must be even -> ones "column" is [P, 2]
        # (memset cannot emit fp32r; stage in f32 and DVE-copy to round)
        ones_f = const.tile([P, P], F32, name="ones_f")
        nc.vector.memset(ones_f, 1.0)
        ones_col2 = const.tile([P, 2], DTM, name="ones_col2")
        nc.vector.tensor_copy(ones_col2, ones_f[:, 0:2])
        ones_row_r = const.tile([1, P], DTM, name="ones_row_r")
        nc.vector.tensor_copy(ones_row_r, ones_f[0:1, :])
        ones_row_f = ones_f[0:1, :]

        # ---- x in (staging pool released after GroupNorm) ----
        xn = data.tile([P, CCH, N], DTM, name="xn")
        resid = data.tile([P, CCH, NQ], F32, name="resid")

        with tc.tile_pool(name="xf_pool", bufs=1) as xf_pool, \
             tc.tile_pool(name="gn_psum", bufs=1, space="PSUM") as gn_psum, \
             tc.tile_pool(name="gn_sb", bufs=1) as gn_sb:
            xf = xf_pool.tile([P, CCH, N], F32, name="xf")
            for ch in range(CCH):
                nc.sync.dma_start(
                    out=xf[:, ch, :], in_=x_d[ch * P:(ch + 1) * P, :]
                )
            # residual (+ bo) for the local query half
            for ch in range(CCH):
                nc.scalar.activation(
                    out=resid[:, ch, :], in_=xf[:, ch, :NQ], func=AF.Identity,
                    bias=bo[ch], scale=1.0,
                )

            # ---- GroupNorm stats ----
            NS = N // 512  # bn_stats subgroups
            pc = []  # per-channel [mean, mean^2 + var] per chunk
            for ch in range(CCH):
                st6 = gn_sb.tile([P, NS, 6], F32, name=f"st6_{ch}")
                for sg in range(NS):
                    nc.vector.bn_stats(
                        out=st6[:, sg, :], in_=xf[:, ch, sg * 512:(sg + 1) * 512]
                    )
                mv = gn_sb.tile([P, 2], F32, name=f"mv{ch}")
                nc.vector.bn_aggr(out=mv, in_=st6)
                pcs = gn_sb.tile([P, 2], F32, name=f"pcs{ch}")
                nc.vector.tensor_copy(pcs[:, 0:1], mv[:, 0:1])
                # pcs[:,1] = mean^2 + var  (-> group E[x^2] after averaging)
                msq = gn_sb.tile([P, 1], F32, name=f"msq{ch}")
                nc.vector.tensor_mul(msq, mv[:, 0:1], mv[:, 0:1])
                nc.vector.tensor_add(pcs[:, 1:2], mv[:, 1:2], msq)
                pc.append(pcs)

            gs_ps = gn_psum.tile([GROUPS, 2], F32, name="gs_ps")
            for ch in range(CCH):
                nc.tensor.matmul(
                    gs_ps, lhsT=gind[ch], rhs=pc[ch],
                    start=(ch == 0), stop=(ch == CCH - 1),
                )
            # per-channel stats are already means -> average over the GSZ
            # channels of each group
            gs = gn_sb.tile([GROUPS, 2], F32, name="gs")
            nc.scalar.mul(gs, gs_ps, 1.0 / GSZ)
            gvar = gn_sb.tile([GROUPS, 1], F32, name="gvar")
            gmsq = gn_sb.tile([GROUPS, 1], F32, name="gmsq")
            nc.vector.tensor_mul(gmsq, gs[:, 0:1], gs[:, 0:1])
            nc.vector.tensor_sub(gvar, gs[:, 1:2], gmsq)
            # rstd = 1/sqrt(var+eps)
            gstd = gn_sb.tile([GROUPS, 1], F32, name="gstd")
            eps_t = gn_sb.tile([GROUPS, 1], F32, name="eps_t")
            nc.vector.memset(eps_t, EPS)
            nc.scalar.activation(
                out=gstd, in_=gvar, func=AF.Sqrt, bias=eps_t, scale=1.0
            )
            gmr = gn_sb.tile([GROUPS, 2], F32, name="gmr")
            nc.vector.tensor_copy(gmr[:, 0:1], gs[:, 0:1])
            nc.vector.reciprocal(gmr[:, 1:2], gstd)

            # broadcast group (mean, rstd) back to channels, build affine
            for ch in range(CCH):
                cb_ps = gn_psum.tile([P, 2], F32, name="cb_ps", tag="cb_ps")
                nc.tensor.matmul(cb_ps, lhsT=gindT[ch], rhs=gmr,
                                 start=True, stop=True)
                cb = gn_sb.tile([P, 2], F32, name=f"cb{ch}")
                nc.vector.tensor_copy(cb, cb_ps)
                scale = gn_sb.tile([P, 1], F32, name=f"scale{ch}")
                nc.vector.tensor_mul(scale, gamma[ch], cb[:, 1:2])
                shift = gn_sb.tile([P, 1], F32, name=f"shift{ch}")
                nc.vector.tensor_mul(shift, cb[:, 0:1], scale)
                nc.vector.tensor_sub(shift, beta[ch], shift)
                # xn = x * scale + shift
                nc.vector.tensor_scalar(
                    out=xn[:, ch, :], in0=xf[:, ch, :],
                    scalar1=scale, scalar2=shift, op0=OP.mult, op1=OP.add,
                )

        # ---- projections ----
        q = data.tile([P, CCH, NQ], DTM, name="q")      # wq^T xn + bq
        qk = data.tile([P, CCH, NQ], DTM, name="qk")    # wk^T q
        vT = data.tile([P, NJT, C], DTM, name="vT")     # v transposed [N, C]

        with tc.tile_pool(name="pj_psum", bufs=3, space="PSUM") as pj_psum:
            # q[o, i] = sum_c wqT[c, o] xn[c, i] + bq[o]
            for oc in range(CCH):
                for it in range(NQ // 512):
                    ps = pj_psum.tile([P, 512], F32, name="q_ps", tag="q_ps")
                    for ch in range(CCH):
                        nc.tensor.matmul(
                            ps,
                            lhsT=wqT[ch][:, oc * P:(oc + 1) * P],
                            rhs=xn[:, ch, it * 512:(it + 1) * 512],
                            start=(ch == 0), stop=(ch == CCH - 1),
                        )
                    nc.vector.tensor_scalar_add(
                        q[:, oc, it * 512:(it + 1) * 512], ps, scalar1=bq[oc]
                    )
            # qk[c', i] = sum_o wk[o, c'] q[o, i]
            for oc in range(CCH):
                for it in range(NQ // 512):
                    ps = pj_psum.tile([P, 512], F32, name="qk_ps", tag="q_ps")
                    for ch in range(CCH):
                        nc.tensor.matmul(
                            ps,
                            lhsT=wk[ch][:, oc * P:(oc + 1) * P],
                            rhs=q[:, ch, it * 512:(it + 1) * 512],
                            start=(ch == 0), stop=(ch == CCH - 1),
                        )
                    nc.vector.tensor_copy(qk[:, oc, it * 512:(it + 1) * 512], ps)
            # vT[j, c] = sum_c' xn[c', j] wvT[c', c] + bv[c]
            for jt in range(NJT):
                ps = pj_psum.tile([P, C], F32, name="vT_ps", tag="vT_ps")
                for ch in range(CCH):
                    nc.tensor.matmul(
                        ps,
                        lhsT=xn[:, ch, jt * P:(jt + 1) * P],
                        rhs=wvT[ch],
                        start=(ch == 0), stop=False,
                    )
                nc.tensor.matmul(
                    ps, lhsT=ones_row_r, rhs=bv_row,
                    start=False, stop=True,
                )
                nc.vector.tensor_copy(vT[:, jt, :], ps)

        # ---- attention ----
        with tc.tile_pool(name="st_psum", bufs=3, space="PSUM") as st_psum, \
             tc.tile_pool(name="o_psum", bufs=1, space="PSUM") as o_psum, \
             tc.tile_pool(name="sm_psum", bufs=1, space="PSUM") as sm_psum, \
             tc.tile_pool(name="at_pool", bufs=4) as at_pool, \
             tc.tile_pool(name="fin", bufs=2) as fin:
            for ib in range(NIB):
                isl = slice(ib * IB, (ib + 1) * IB)
                sums_ps = sm_psum.tile([2, IB], F32, name="sums_ps", tag="sums")
                o_ps = [
                    o_psum.tile([P, IB], F32, name=f"o_ps{cc}", tag=f"o{cc}")
                    for cc in range(CCH)
                ]
                for jt in range(NJT):
                    jsl = slice(jt * P, (jt + 1) * P)
                    st = st_psum.tile([P, IB], F32, name="st", tag="st")
                    # S^T[j, i] = sum_c' xn[c', j] qk[c', i]
                    for ch in range(CCH):
                        nc.tensor.matmul(
                            st,
                            lhsT=xn[:, ch, jsl],
                            rhs=qk[:, ch, isl],
                            start=(ch == 0), stop=(ch == CCH - 1),
                        )
                    at = at_pool.tile([P, IB], DTM, name="at", tag="at")
                    # A^T = exp(S^T / sqrt(C))
                    nc.scalar.activation(
                        out=at, in_=st, func=AF.Exp, scale=1.0 / 16.0
                    )
                    # denominator: column sums via ones-matmul (row 0; row 1
                    # is a duplicate forced by the even-free-dim rule)
                    nc.tensor.matmul(
                        sums_ps, lhsT=ones_col2, rhs=at,
                        start=(jt == 0), stop=(jt == NJT - 1),
                    )
                    # O[c, i] += v^T[j, c]^T A^T[j, i]
                    for cc in range(CCH):
                        nc.tensor.matmul(
                            o_ps[cc],
                            lhsT=vT[:, jt, cc * P:(cc + 1) * P],
                            rhs=at,
                            start=(jt == 0), stop=(jt == NJT - 1),
                        )

                recip = fin.tile([1, IB], F32, name="recip", tag="recip")
                nc.vector.reciprocal(recip, sums_ps[0:1, :])
                rb_ps = sm_psum.tile([P, IB], F32, name="rb_ps", tag="rb")
                nc.tensor.matmul(rb_ps, lhsT=ones_row_f, rhs=recip,
                                 start=True, stop=True)
                rb = fin.tile([P, IB], F32, name="rb", tag="rbs")
                nc.scalar.copy(rb, rb_ps)

                o_sb = []
                for cc in range(CCH):
                    t = fin.tile([P, IB], DTM, name=f"o_sb{cc}", tag=f"osb{cc}")
                    nc.vector.tensor_copy(t, o_ps[cc])
                    o_sb.append(t)

                for oc in range(CCH):
                    op_ps = st_psum.tile([P, IB], F32, name="op_ps", tag="st")
                    for cc in range(CCH):
                        nc.tensor.matmul(
                            op_ps,
                            lhsT=woT[cc][:, oc * P:(oc + 1) * P],
                            rhs=o_sb[cc],
                            start=(cc == 0), stop=(cc == CCH - 1),
                        )
                    t = fin.tile([P, IB], F32, name="t_sb", tag="t_sb")
                    nc.vector.tensor_mul(t, op_ps, rb)
                    out_sb = fin.tile([P, IB], F32, name="out_sb", tag="out_sb")
                    nc.vector.tensor_add(out_sb, t, resid[:, oc, isl])
                    nc.sync.dma_start(
                        out=y_d[oc * P:(oc + 1) * P, isl], in_=out_sb
                    )
    nc.finalize()
    return nc


_NC_CACHE = {}


def _get_nc(mm_fast=True):
    key = mm_fast
    if key not in _NC_CACHE:
        _NC_CACHE[key] = build_nc(mm_fast)
    return _NC_CACHE[key]


def make_in_maps(inputs):
    """Shard full inputs into per-core input maps."""
    x = np.asarray(inputs["x"], np.float32).reshape(B, C, N)
    gamma = np.asarray(inputs["gamma"], np.float32)
    beta = np.asarray(inputs["beta"], np.float32)
    wq = np.asarray(inputs["wq"], np.float32)
    bq = np.asarray(inputs["bq"], np.float32)
    wk = np.asarray(inputs["wk"], np.float32)
    wv = np.asarray(inputs["wv"], np.float32)
    bv = np.asarray(inputs["bv"], np.float32)
    wo = np.asarray(inputs["wo"], np.float32)
    bo = np.asarray(inputs["bo"], np.float32)

    gind = np.zeros((CCH, P, GROUPS), np.float32)
    for ch in range(CCH):
        for p in range(P):
            gind[ch, p, (ch * P + p) // GSZ] = 1.0
    gindT = np.ascontiguousarray(gind.transpose(0, 2, 1))

    shared = {
        "wqT": np.ascontiguousarray(wq.T),
        "wk": np.ascontiguousarray(wk),
        "wvT": np.ascontiguousarray(wv.T),
        "woT": np.ascontiguousarray(wo.T),
        "gamma": gamma, "beta": beta,
        "bq": bq, "bv": bv, "bo": bo,
        "gind": gind, "gindT": gindT,
    }
    in_maps = []
    for core in range(NCORES):
        b, h = divmod(core, QSPLIT)
        if h == 0:
            xc = x[b]
        else:
            xc = np.concatenate(
                [x[b][:, h * NQ:(h + 1) * NQ], x[b][:, :h * NQ],
                 x[b][:, (h + 1) * NQ:]], axis=1,
            )
        in_maps.append({"x": np.ascontiguousarray(xc), **shared})
    return in_maps


def gather_output(results):
    y = np.empty((B, C, N), np.float32)
    for core in range(NCORES):
        b, h = divmod(core, QSPLIT)
        y[b][:, h * NQ:(h + 1) * NQ] = results[core]["y"]
    return y.reshape(B, C, H, W)


def _run_traced(nc, in_maps, core_ids, tmpdir=None):
    """Replicates run_bass_kernel_spmd's axon trace branch; this image
    lacks antenv.axon_hooks, so drive the NTFF hook via ctypes directly."""
    import glob
    import tempfile

    import gauge.profiler
    from concourse import bass2jax
    from concourse._compat import FishPath
    from concourse.bass_utils import BassKernelResults, _process_ntff_profile
    from trn_agent_boot.trn_boot import _ntff_profile_via_ctypes

    hook = _ntff_profile_via_ctypes("/opt/axon/libaxon_pjrt.so")
    if tmpdir is None:
        tmpdir = tempfile.mkdtemp(prefix="bassprof_")
    if hook is None:
        results = bass2jax.run_bass_via_pjrt(nc, in_maps, n_cores=len(core_ids))
        return BassKernelResults(results, None, None, None)
    with hook(tmpdir, [0]):
        results = bass2jax.run_bass_via_pjrt(nc, in_maps, n_cores=len(core_ids))
    if not glob.glob(f"{tmpdir}/*_body*.ntff"):
        print(f"no NTFF produced in {tmpdir}")
        return BassKernelResults(results, None, None, None)
    profile = gauge.profiler.Profile(
        profile_path=FishPath(tmpdir),
        kernel_dev_mode=True,
        profile_on_exit=False,
        bass_kernel=nc.m,
        offline_processing=True,
        fname="*_body*",
        metadata={},
    )
    return _process_ntff_profile(
        profile, tmpdir, nc, core_ids, None, False, {}, False
    ).as_bass_kernel_results(results)


def run_spmd(inputs, trace=False, mm_fast=True, tmpdir=None):
    from concourse.bass_utils import run_bass_kernel_spmd

    nc = _get_nc(mm_fast)
    in_maps = make_in_maps(inputs)
    if trace:
        res = _run_traced(nc, in_maps, list(range(NCORES)), tmpdir=tmpdir)
    else:
        res = run_bass_kernel_spmd(nc, in_maps, list(range(NCORES)), trace=False)
    return gather_output(res.results), res


def kernel(**inputs) -> np.ndarray:
    out, _ = run_spmd(inputs, trace=False)
    return out
